# revision 1
# baseline (speedup 1.0000x reference)
"""MoE kernel for Trainium2 (8 NeuronCores), expert-parallel.

Strategy:
  - Host computes the (tiny) router: logits = x @ router_w in f64, softmax,
    top-2 expert indices + gate probs per token (verified to match
    jax.lax.top_k selection exactly on f32 ties-by-lower-index).
  - Tokens are gathered per routed expert on host (all-to-all dispatch done
    at input-sharding time). Core e receives its expert's tokens padded to
    capacity C (max expert load rounded to 128).
  - The shared expert is split along the FFN dim F: core e owns columns
    [e*512,(e+1)*512) of S_up and the matching rows of S_down, and computes
    a partial shared output for ALL tokens; the host sums the 8 partials
    (a sum over F-slices is exact in the FFN structure since only gelu is
    nonlinear and it is applied per-F-element before the down projection).
  - Device kernel per core, two phases with all weights SBUF-resident:
      phase S: partial shared FFN over all 8192 tokens (F-slice 512)
      phase R: own routed expert over C gathered tokens, gate fused into
               the PSUM eviction
    Matmuls in bf16 with f32 PSUM accumulation; exact-erf gelu on ScalarE.
    Phase S weights are tiny (4MB) so compute starts almost immediately;
    the 16MB routed weights stream in on the SWDGE queue behind it.
  - Host combines: y = x + sum_cores shared_partial + gather of gated
    routed outputs (each token's top-2 expert rows).
"""

import sys

if "/opt/trn_rl_repo" not in sys.path:
    sys.path.insert(0, "/opt/trn_rl_repo")

from contextlib import ExitStack

import ml_dtypes
import numpy as np

H, F, E, TOPK = 1024, 4096, 8, 2
N_CORES = 8
CHUNK = 256  # tokens per pipeline chunk (2 c-tiles of 128)
NOUT = 2  # h-output tiles of 512
FS = F // N_CORES  # shared-expert F-slice per core (512)
BF16 = ml_dtypes.bfloat16

_nc_cache = {}

# test-harness hooks (unused when graded): set TRACE=True to request an NTFF
# profile; the BassKernelResults of the last run lands in LAST_RESULT.
TRACE = False
LAST_RESULT = None


def _ffn_phase(nc, tile, dt, act, *, wu, wd, x_r, out_r, c_lo, c_hi, n_f,
               pools, g_sb=None, g_base=0, paced_dmas=None):
    """One dense FFN phase: out = [gate *] gelu(x @ Wup) @ Wdown.

    wu: list of k-tiles [128, n_f*128] (lhsT slices along H)
    wd: list of n_f tiles [128, H]
    x_r/out_r: DRAM APs [128, kt, tokens] / [128, tokens/128, H]
    """
    import concourse.mybir as mybir

    xpool, hpool, opool, pup, pdown = pools
    KT_H = H // 128
    GELU = getattr(mybir.ActivationFunctionType, act)

    n_chunks = -(-(c_hi - c_lo) // CHUNK)
    for ic, c0 in enumerate(range(c_lo, c_hi, CHUNK)):
        cc = min(CHUNK, c_hi - c0)
        nct = cc // 128
        x_sb = xpool.tile([128, KT_H, CHUNK], dt.bfloat16, tag="x")
        x_dma = nc.sync.dma_start(x_sb[:, :, :cc], x_r[:, :, c0 : c0 + cc])
        # one single-bank PSUM tile per (ci,ho) output slice: gives each
        # slice its own semaphore, so evictions start as soon as that
        # slice's accumulation stops and the next chunk's first down
        # matmuls wait only on their own slice's eviction.
        ps_d = [
            pdown.tile([128, 512], dt.float32, tag=f"pd{s}", name=f"pd{s}")
            for s in range(nct * NOUT)
        ]
        if paced_dmas:
            # pace bulk background DMAs (next phase's weights) across this
            # phase: emit a slice per chunk, gated on this chunk's x arrival
            # so they don't hog HBM bandwidth ahead of the compute stream.
            from concourse.bass import _add_dep_helper

            # skip the first chunks entirely: they prime the compute pipeline
            # and any HBM contention there stalls the PE directly
            skip = min(2, n_chunks - 1)
            span = n_chunks - skip
            lo = len(paced_dmas) * max(0, ic - skip) // span
            hi = len(paced_dmas) * max(0, ic - skip + 1) // span
            for fn in paced_dmas[lo:hi]:
                w_dma = fn()
                _add_dep_helper(
                    w_dma.ins, x_dma.ins, True, "paced background weight DMA"
                )
        # f-loop pipelined by one step: down(f) is emitted after up(f+1) so
        # the gelu -> LDWEIGHTS(hT) chain of step f hides under the up
        # matmuls of step f+1 instead of stalling the first down matmul.
        def emit_up(f):
            ps_u = pup.tile([128, cc], dt.float32, tag="pu")
            for kt in range(KT_H):
                nc.tensor.matmul(
                    ps_u[:],
                    wu[kt][:, f * 128 : (f + 1) * 128],
                    x_sb[:, kt, :cc],
                    start=(kt == 0),
                    stop=(kt == KT_H - 1),
                )
            hT = hpool.tile([128, cc], dt.bfloat16, tag="h")
            nc.scalar.activation(hT[:], ps_u[:], GELU)
            return hT

        def emit_down(f, hT):
            for ci in range(nct):
                for ho in range(NOUT):
                    nc.tensor.matmul(
                        ps_d[ci * NOUT + ho][:],
                        hT[:, ci * 128 : (ci + 1) * 128],
                        wd[f][:, ho * 512 : (ho + 1) * 512],
                        start=(f == 0),
                        stop=(f == n_f - 1),
                    )

        depth = 2 if n_f > 2 else 1
        hts = [emit_up(f) for f in range(min(depth, n_f))]
        for f in range(depth, n_f):
            hts.append(emit_up(f))
            emit_down(f - depth, hts[f - depth])
        for f in range(max(0, n_f - depth), n_f):
            emit_down(f, hts[f])

        for ci in range(nct):
            n = (c0 - c_lo) // 128 + ci
            o_sb = opool.tile([128, H], dt.float32, tag="o")
            for ho in range(NOUT):
                dst = o_sb[:, ho * 512 : (ho + 1) * 512]
                src = ps_d[ci * NOUT + ho][:]
                # split evictions across DVE and ACT (Copy/Identity share the
                # gelu PWP table set, so no table reload) — halves the
                # eviction latency the next chunk's down matmuls wait on
                if g_sb is not None:
                    g = g_sb[:, g_base + n : g_base + n + 1]
                    if ho % 2 == 0:
                        nc.vector.tensor_scalar_mul(dst, src, g)
                    else:
                        nc.scalar.activation(
                            dst, src, mybir.ActivationFunctionType.Copy, scale=g
                        )
                else:
                    if ho % 2 == 0:
                        nc.vector.tensor_copy(dst, src)
                    else:
                        nc.scalar.activation(
                            dst, src, mybir.ActivationFunctionType.Copy
                        )
            nc.sync.dma_start(out_r[:, n, :], o_sb[:])


def _build_nc(c_routed, t_total, act="Gelu"):
    import concourse.mybir as mybir
    import concourse.tile as tile
    from concourse import bacc

    dt = mybir.dt
    assert c_routed % 128 == 0 and t_total % CHUNK == 0
    KT_H = H // 128  # 8 k-tiles along H
    KT_F = F // 128  # 32 k-tiles along F (routed down-proj)
    NF_S = FS // 128  # 4 f-tiles in the shared slice

    # Bacc (not raw Bass): its compile pass splits sync waits down to the
    # TRN2 limit of 1 wait per instruction (walrus rejects multi-wait IR).
    nc = bacc.Bacc(None, target_bir_lowering=False)
    xT_r = nc.dram_tensor("xT_r", [H, c_routed], dt.bfloat16, kind="ExternalInput")
    xT_s = nc.dram_tensor("xT_s", [H, t_total], dt.bfloat16, kind="ExternalInput")
    gates = nc.dram_tensor(
        "gates", [128, c_routed // 128], dt.float32, kind="ExternalInput"
    )
    w_up = nc.dram_tensor("w_up", [H, F], dt.bfloat16, kind="ExternalInput")
    w_down = nc.dram_tensor("w_down", [F, H], dt.bfloat16, kind="ExternalInput")
    su_s = nc.dram_tensor("su_s", [H, FS], dt.bfloat16, kind="ExternalInput")
    sd_s = nc.dram_tensor("sd_s", [FS, H], dt.bfloat16, kind="ExternalInput")
    out_r = nc.dram_tensor("out_r", [c_routed, H], dt.float32, kind="ExternalOutput")
    out_s = nc.dram_tensor("out_s", [t_total, H], dt.float32, kind="ExternalOutput")

    xTr_t = xT_r.rearrange("(kt p) c -> p kt c", p=128)
    xTs_t = xT_s.rearrange("(kt p) c -> p kt c", p=128)
    outr_t = out_r.rearrange("(n p) h -> p n h", p=128)
    outs_t = out_s.rearrange("(n p) h -> p n h", p=128)

    with tile.TileContext(nc) as tc, ExitStack() as ctx:
        swpool = ctx.enter_context(tc.tile_pool(name="sweights", bufs=1))
        wpool = ctx.enter_context(tc.tile_pool(name="weights", bufs=1))
        xpool = ctx.enter_context(tc.tile_pool(name="x", bufs=3))
        hpool = ctx.enter_context(tc.tile_pool(name="h", bufs=6))
        cpool = ctx.enter_context(tc.tile_pool(name="const", bufs=1))
        opool = ctx.enter_context(tc.tile_pool(name="out", bufs=3))
        # 4 psd slices + 3 pup bufs = 7 of 8 PSUM banks; bufs=4 (all 8 banks)
        # crashes the device (NRT_EXEC_UNIT_UNRECOVERABLE) — do not fill PSUM.
        pup = ctx.enter_context(tc.tile_pool(name="pup", bufs=3, space="PSUM"))
        pdown = ctx.enter_context(tc.tile_pool(name="pdown", bufs=1, space="PSUM"))
        pools = (xpool, hpool, opool, pup, pdown)

        # shared-slice weights (small, on the HWDGE queue -> available fast);
        # one coalesced DMA each so SP-sequencer dispatch doesn't delay the
        # first x-chunk load behind a dozen small descriptors
        su_all = swpool.tile([128, KT_H, FS], dt.bfloat16, tag="su")
        nc.sync.dma_start(su_all[:], su_s.rearrange("(kt p) f -> p kt f", p=128)[:])
        su = [su_all[:, kt, :] for kt in range(KT_H)]
        sd_all = swpool.tile([128, NF_S, H], dt.bfloat16, tag="sd")
        nc.sync.dma_start(sd_all[:], sd_s.rearrange("(ft p) h -> p ft h", p=128)[:])
        sd = [sd_all[:, ft, :] for ft in range(NF_S)]

        # routed weights (16MB): tiles allocated now, DMAs deferred — they
        # are emitted paced across the shared phase (on the SWDGE queue) so
        # they don't steal HBM bandwidth from the shared phase's startup.
        wu, wd, w_dma_fns = [], [], []
        wu_t = w_up.rearrange("(kt p) f -> p kt f", p=128)
        for kt in range(KT_H):
            t = wpool.tile([128, F], dt.bfloat16, tag=f"wu{kt}")
            w_dma_fns.append(
                lambda t=t, kt=kt: nc.gpsimd.dma_start(t[:], wu_t[:, kt, :])
            )
            wu.append(t)
        wd_t = w_down.rearrange("(ft p) h -> p ft h", p=128)
        for ft in range(KT_F):
            t = wpool.tile([128, H], dt.bfloat16, tag=f"wd{ft}")
            w_dma_fns.append(
                lambda t=t, ft=ft: nc.gpsimd.dma_start(t[:], wd_t[:, ft, :])
            )
            wd.append(t)

        # phase S: partial shared FFN over all tokens, F-slice FS
        _ffn_phase(nc, tile, dt, act, wu=su, wd=sd, x_r=xTs_t, out_r=outs_t,
                   c_lo=0, c_hi=t_total, n_f=NF_S, pools=pools,
                   paced_dmas=w_dma_fns)

        g_sb = cpool.tile([128, c_routed // 128], dt.float32)
        nc.sync.dma_start(g_sb[:], gates[:])
        # phase R: routed expert over gathered tokens, gated eviction
        _ffn_phase(nc, tile, dt, act, wu=wu, wd=wd, x_r=xTr_t, out_r=outr_t,
                   c_lo=0, c_hi=c_routed, n_f=KT_F, pools=pools,
                   g_sb=g_sb)

    nc.finalize()
    return nc


def _get_nc(c_routed, t_total):
    key = (c_routed, t_total)
    if key not in _nc_cache:
        _nc_cache[key] = _build_nc(c_routed, t_total)
    return _nc_cache[key]


def _route(xf, router_w):
    """Host router in f64: top-2 indices (jax tie-break: lower index first)
    and their softmax probs."""
    logits = xf.astype(np.float64) @ router_w.astype(np.float64)
    m = logits.max(-1, keepdims=True)
    p = np.exp(logits - m)
    p /= p.sum(-1, keepdims=True)
    order = np.argsort(-p, axis=-1, kind="stable")
    top_idx = order[:, :TOPK]
    top_p = np.take_along_axis(p, top_idx, -1).astype(np.float32)
    return top_idx, top_p


def kernel(**inputs):
    x = np.ascontiguousarray(np.asarray(inputs["x"], np.float32))
    shared_up = np.asarray(inputs["shared_up"], np.float32)[0]
    shared_down = np.asarray(inputs["shared_down"], np.float32)[0]
    routed_up = np.asarray(inputs["routed_up"], np.float32)
    routed_down = np.asarray(inputs["routed_down"], np.float32)
    router_w = np.asarray(inputs["router_w"], np.float32)

    B, S, _ = x.shape
    T = B * S
    xf = x.reshape(T, H)

    top_idx, top_p = _route(xf, router_w)

    token_lists = [np.where((top_idx == e).any(-1))[0] for e in range(E)]
    c_cap = max(128, -(-max(len(l) for l in token_lists) // 128) * 128)

    # position of (token, slot) inside its expert's gathered buffer
    pos = np.zeros((T, TOPK), np.int64)
    gates_per_e = np.zeros((E, c_cap), np.float32)
    for e in range(E):
        lst = token_lists[e]
        for k in range(TOPK):
            sel = np.where(top_idx[:, k] == e)[0]
            p_in = np.searchsorted(lst, sel)
            pos[sel, k] = p_in
            gates_per_e[e, p_in] = top_p[sel, k]

    xf_bf = xf.astype(BF16)
    xTs = np.ascontiguousarray(xf_bf.T)  # [H, T], shared phase input
    su_bf = shared_up.astype(BF16)
    sd_bf = shared_down.astype(BF16)

    in_maps = []
    for e in range(E):
        lst = token_lists[e]
        xe = np.zeros((c_cap, H), BF16)
        xe[: len(lst)] = xf_bf[lst]
        in_maps.append(
            {
                "xT_r": np.ascontiguousarray(xe.T),
                "xT_s": xTs,
                "gates": np.ascontiguousarray(
                    gates_per_e[e].reshape(c_cap // 128, 128).T
                ),
                "w_up": routed_up[e].astype(BF16),
                "w_down": routed_down[e].astype(BF16),
                "su_s": np.ascontiguousarray(su_bf[:, e * FS : (e + 1) * FS]),
                "sd_s": np.ascontiguousarray(sd_bf[e * FS : (e + 1) * FS, :]),
            }
        )

    from concourse.bass_utils import run_bass_kernel_spmd

    nc = _get_nc(c_cap, T)
    res = run_bass_kernel_spmd(nc, in_maps, list(range(N_CORES)), trace=TRACE)
    global LAST_RESULT
    LAST_RESULT = res

    y = xf.copy()
    for e in range(E):
        y += res.results[e]["out_s"]
    y_routed = np.stack([res.results[e]["out_r"] for e in range(E)])  # gated rows
    for k in range(TOPK):
        y += y_routed[top_idx[:, k], pos[:, k]]
    return y.reshape(B, S, H)



# revision 6
# speedup vs baseline: 1.6858x; 1.6858x over previous
"""MoE kernel for Trainium2 (8 NeuronCores), expert-parallel.

Strategy:
  - Host computes the (tiny) router: logits = x @ router_w in f64, softmax,
    top-2 expert indices + gate probs per token (verified to match
    jax.lax.top_k selection exactly on f32 ties-by-lower-index).
  - Tokens are gathered per routed expert on host (all-to-all dispatch done
    at input-sharding time). Core e receives its expert's tokens padded to
    capacity C (max expert load rounded to 128).
  - The shared expert is split along the FFN dim F: core e owns columns
    [e*512,(e+1)*512) of S_up and the matching rows of S_down, and computes
    a partial shared output for ALL tokens; the host sums the 8 partials
    (a sum over F-slices is exact in the FFN structure since only gelu is
    nonlinear and it is applied per-F-element before the down projection).
  - Device kernel per core, two phases with all weights SBUF-resident:
      phase S: partial shared FFN over all 8192 tokens (F-slice 512);
               up-proj in bf16 (its error dominates the output, so it stays
               high precision), down-proj in fp8 DoubleRow
      phase R: own routed expert over C gathered tokens, fully fp8
               DoubleRow (2 k-tiles contracted per pass, 2x PE rate);
               512-token chunks so the 512-col up matmuls hide their
               256-row LDWEIGHTS; gate fused into the PSUM eviction.
    f32 PSUM accumulation everywhere; exact-erf gelu on ScalarE.
    fp8 scaling: routed x pre-scaled by 16, all fp8 weights by 256; the
    routed gelu applies scale=1/4096 to recover the exact pre-activation.
    The routed gates are pre-divided by 256 on host; the shared partials
    come back scaled by 256 and the host divides once after summing.
  - Host combines: y = x + sum_cores shared_partial/256 + gather of gated
    routed outputs (each token's top-2 expert rows).
"""

import sys

if "/opt/trn_rl_repo" not in sys.path:
    sys.path.insert(0, "/opt/trn_rl_repo")

from contextlib import ExitStack

import ml_dtypes
import numpy as np

H, F, E, TOPK = 1024, 4096, 8, 2
N_CORES = 8
NOUT = 2  # h-output tiles of 512
FS = F // N_CORES  # shared-expert F-slice per core (512)
BF16 = ml_dtypes.bfloat16
FP8 = ml_dtypes.float8_e4m3  # TRN variant: max normal 240
SX = 16.0  # fp8 x scale
SW = 256.0  # fp8 weight scale

_nc_cache = {}

# test-harness hooks (unused when graded): set TRACE=True to request an NTFF
# profile; the BassKernelResults of the last run lands in LAST_RESULT.
TRACE = False
LAST_RESULT = None


def _chunk_sizes(c_hi, chunk):
    """Chunk layout with every chunk >=256 tokens when possible, so the
    up matmuls (cc cols) always cover their 256-row DoubleRow LDWEIGHTS.
    A 128-token remainder is folded into the last two chunks (384+256)."""
    if c_hi <= chunk:
        return [c_hi]
    sizes = []
    rem = c_hi
    while rem > chunk + 128:
        sizes.append(chunk)
        rem -= chunk
    if rem <= chunk:
        sizes.append(rem)
    else:  # rem in (chunk, chunk+128]: split >=256 each
        sizes += [rem - 256, 256]
    return sizes


def _ffn_phase(nc, tile, dt, act, *, up_fp8, wu, wd_all, x_r, out_r, c_hi,
               n_f, pools, chunk, act_scale=1.0, g_sb=None, paced_dmas=None,
               front_dmas=None, x0_pre=None):
    """One FFN phase: out = [gate *] gelu(x @ Wup) @ Wdown.

    Up-proj: bf16 (wu = list of KT_H k-tiles [128, n_f*128]) or fp8
    DoubleRow (wu = sbuf tile [128, KT_H, n_f*128], x fp8).
    Down-proj: always fp8 DoubleRow; wd_all = sbuf tile [128, n_f, H] with
    f-tiles along dim 1 so consecutive pairs form the DoubleRow operand.
    The gelu writes fp8 h-pairs [128, 2, cc] (the down stationary operand).
    Chunks wider than 256 run the down matmuls in multiple passes over
    ci-pairs, reusing the same 4 PSUM tiles (pup 3 + pdown 4 = 7 banks).
    x_r/out_r: DRAM APs [128, kt, tokens] / [128, tokens/128, H].
    front_dmas: emitted right after chunk 0's x DMA (startup interleave).
    x0_pre: optional preloaded x tile for chunk 0 (prefetched last phase).
    """
    import concourse.mybir as mybir

    xpool, hpool, opool, pup, pdown = pools
    KT_H = H // 128
    NKP = KT_H // 2  # k-tile pairs for the fp8 up matmul
    NFP = n_f // 2  # f-tile pairs for the down matmul
    GELU = getattr(mybir.ActivationFunctionType, act)
    COPY = mybir.ActivationFunctionType.Copy
    DR = mybir.MatmulPerfMode.DoubleRow
    x_dt = dt.float8e4 if up_fp8 else dt.bfloat16

    sizes = _chunk_sizes(c_hi, chunk)
    n_chunks = len(sizes)
    starts = [sum(sizes[:i]) for i in range(n_chunks)]
    for ic, (c0, cc) in enumerate(zip(starts, sizes)):
        nct = cc // 128
        if ic == 0 and x0_pre is not None:
            x_sb = x0_pre
        else:
            x_sb = xpool.tile([128, KT_H, chunk], x_dt, tag="x")
            x_dma = nc.sync.dma_start(x_sb[:, :, :cc], x_r[:, :, c0 : c0 + cc])
        if ic == 0 and front_dmas:
            for fn in front_dmas:
                fn()
        if paced_dmas:
            # pace bulk background DMAs (next phase's weights) across this
            # phase: emit a slice per chunk, gated on this chunk's x arrival
            # so they don't hog HBM bandwidth ahead of the compute stream.
            from concourse.bass import _add_dep_helper

            # skip the first chunks entirely: they prime the compute pipeline
            # and any HBM contention there stalls the PE directly
            skip = min(2, n_chunks - 1)
            span = n_chunks - skip
            lo = len(paced_dmas) * max(0, ic - skip) // span
            hi = len(paced_dmas) * max(0, ic - skip + 1) // span
            for fn in paced_dmas[lo:hi]:
                w_dma = fn()
                _add_dep_helper(
                    w_dma.ins, x_dma.ins, True, "paced background weight DMA"
                )

        # f-tiles processed in pairs: both gelu outputs of a pair land in one
        # [128, 2, cc] fp8 tile, which is the DoubleRow stationary operand of
        # the down matmul (contracts both f-tiles at once). The pair loop is
        # pipelined one pair deep so each gelu -> LDWEIGHTS(h) chain hides
        # under the next pair's up matmuls.
        def emit_up(f, hp):
            ps_u = pup.tile([128, cc], dt.float32, tag="pu")
            if up_fp8:
                for kp in range(NKP):
                    nc.tensor.matmul(
                        ps_u[:],
                        wu[:, 2 * kp : 2 * kp + 2, f * 128 : (f + 1) * 128],
                        x_sb[:, 2 * kp : 2 * kp + 2, :cc],
                        start=(kp == 0),
                        stop=(kp == NKP - 1),
                        perf_mode=DR,
                    )
            else:
                for kt in range(KT_H):
                    nc.tensor.matmul(
                        ps_u[:],
                        wu[kt][:, f * 128 : (f + 1) * 128],
                        x_sb[:, kt, :cc],
                        start=(kt == 0),
                        stop=(kt == KT_H - 1),
                    )
            nc.scalar.activation(hp[:, f % 2, :cc], ps_u[:], GELU,
                                 scale=act_scale)

        def emit_up_pair(j):
            hp = hpool.tile([128, 2, chunk], dt.float8e4, tag="h")
            emit_up(2 * j, hp)
            emit_up(2 * j + 1, hp)
            return hp

        def emit_down_pair(j, hp, cis, ps_d):
            for slot, ci in enumerate(cis):
                for ho in range(NOUT):
                    nc.tensor.matmul(
                        ps_d[slot * NOUT + ho][:],
                        hp[:, :, ci * 128 : (ci + 1) * 128],
                        wd_all[:, 2 * j : 2 * j + 2, ho * 512 : (ho + 1) * 512],
                        start=(j == 0),
                        stop=(j == NFP - 1),
                        perf_mode=DR,
                    )

        def evict(cis, ps_d, last):
            for slot, ci in enumerate(cis):
                n = c0 // 128 + ci
                o_sb = opool.tile([128, H], dt.bfloat16, tag="o")
                g = g_sb[:, n : n + 1] if g_sb is not None else None
                # split evictions across DVE and ACT (Copy shares the gelu
                # PWP table set, so no table reload); the kernel's last
                # eviction splits 256-col pieces to shorten the tail.
                w = 256 if last else 512
                for ho in range(NOUT):
                    for o0 in range(ho * 512, (ho + 1) * 512, w):
                        dst = o_sb[:, o0 : o0 + w]
                        src = ps_d[slot * NOUT + ho][:, o0 - ho * 512 :
                                                     o0 - ho * 512 + w]
                        eng = (o0 // w) % 2 == 0
                        if g is not None:
                            if eng:
                                nc.vector.tensor_scalar_mul(dst, src, g)
                            else:
                                nc.scalar.activation(dst, src, COPY, scale=g)
                        else:
                            if eng:
                                nc.vector.tensor_copy(dst, src)
                            else:
                                nc.scalar.activation(dst, src, COPY)
                nc.sync.dma_start(out_r[:, n, :], o_sb[:])

        ci_passes = [list(range(p, min(p + 2, nct))) for p in range(0, nct, 2)]
        is_last_chunk = ic == n_chunks - 1

        # pass 0 pipelined against the up pairs
        ps_d = [
            pdown.tile([128, 512], dt.float32, tag=f"pd{s}", name=f"pd{s}")
            for s in range(len(ci_passes[0]) * NOUT)
        ]
        depth = 1  # one pair (= 2 f-tiles) of lookahead
        hps = [emit_up_pair(j) for j in range(min(depth, NFP))]
        for j in range(depth, NFP):
            hps.append(emit_up_pair(j))
            emit_down_pair(j - depth, hps[j - depth], ci_passes[0], ps_d)
        for j in range(max(0, NFP - depth), NFP):
            emit_down_pair(j, hps[j], ci_passes[0], ps_d)
        evict(ci_passes[0], ps_d, is_last_chunk and len(ci_passes) == 1)

        # remaining ci passes reuse the h pairs (and the freed PSUM tiles)
        for pi, cis in enumerate(ci_passes[1:], 1):
            ps_d = [
                pdown.tile([128, 512], dt.float32, tag=f"pd{s}", name=f"pd{s}")
                for s in range(len(cis) * NOUT)
            ]
            for j in range(NFP):
                emit_down_pair(j, hps[j], cis, ps_d)
            evict(cis, ps_d, is_last_chunk and pi == len(ci_passes) - 1)


def _build_nc(c_routed, t_total, act="Gelu"):
    import concourse.mybir as mybir
    import concourse.tile as tile
    from concourse import bacc

    dt = mybir.dt
    assert c_routed % 128 == 0 and t_total % 256 == 0
    KT_H = H // 128  # 8 k-tiles along H
    KT_F = F // 128  # 32 k-tiles along F (routed down-proj)
    NF_S = FS // 128  # 4 f-tiles in the shared slice

    # Bacc (not raw Bass): its compile pass splits sync waits down to the
    # TRN2 limit of 1 wait per instruction (walrus rejects multi-wait IR).
    nc = bacc.Bacc(None, target_bir_lowering=False)
    xT_r = nc.dram_tensor("xT_r", [H, c_routed], dt.float8e4, kind="ExternalInput")
    xT_s = nc.dram_tensor("xT_s", [H, t_total], dt.bfloat16, kind="ExternalInput")
    gates = nc.dram_tensor(
        "gates", [128, c_routed // 128], dt.float32, kind="ExternalInput"
    )
    w_up = nc.dram_tensor("w_up", [H, F], dt.float8e4, kind="ExternalInput")
    w_down = nc.dram_tensor("w_down", [F, H], dt.float8e4, kind="ExternalInput")
    su_s = nc.dram_tensor("su_s", [H, FS], dt.bfloat16, kind="ExternalInput")
    sd_s = nc.dram_tensor("sd_s", [FS, H], dt.float8e4, kind="ExternalInput")
    out_r = nc.dram_tensor("out_r", [c_routed, H], dt.bfloat16, kind="ExternalOutput")
    out_s = nc.dram_tensor("out_s", [t_total, H], dt.bfloat16, kind="ExternalOutput")

    xTr_t = xT_r.rearrange("(kt p) c -> p kt c", p=128)
    xTs_t = xT_s.rearrange("(kt p) c -> p kt c", p=128)
    outr_t = out_r.rearrange("(n p) h -> p n h", p=128)
    outs_t = out_s.rearrange("(n p) h -> p n h", p=128)

    with tile.TileContext(nc) as tc, ExitStack() as ctx:
        swpool = ctx.enter_context(tc.tile_pool(name="sweights", bufs=1))
        wpool = ctx.enter_context(tc.tile_pool(name="weights", bufs=1))
        xpool = ctx.enter_context(tc.tile_pool(name="x", bufs=3))
        hpool = ctx.enter_context(tc.tile_pool(name="h", bufs=20))
        cpool = ctx.enter_context(tc.tile_pool(name="const", bufs=1))
        opool = ctx.enter_context(tc.tile_pool(name="out", bufs=3))
        # 4 psd slices + 3 pup bufs = 7 of 8 PSUM banks; bufs=4 (all 8 banks)
        # crashes the device (NRT_EXEC_UNIT_UNRECOVERABLE) — do not fill PSUM.
        pup = ctx.enter_context(tc.tile_pool(name="pup", bufs=3, space="PSUM"))
        pdown = ctx.enter_context(tc.tile_pool(name="pdown", bufs=1, space="PSUM"))
        pools = (xpool, hpool, opool, pup, pdown)

        su_t = su_s.rearrange("(kt p) f -> p kt f", p=128)
        su_all = swpool.tile([128, KT_H, FS], dt.bfloat16, tag="su")
        # f-column slices: f0's weights (needed by the very first matmul)
        # land first; f1..f3 stream in behind chunk 0's x (front_dmas).
        nc.sync.dma_start(su_all[:, :, 0:128], su_t[:, :, 0:128])
        su_front = [
            lambda f=f: nc.sync.dma_start(
                su_all[:, :, f * 128 : (f + 1) * 128],
                su_t[:, :, f * 128 : (f + 1) * 128],
            )
            for f in range(1, NF_S)
        ]
        su = [su_all[:, kt, :] for kt in range(KT_H)]
        # shared down weights (fp8, f-tiles along dim 1): on the SWDGE queue
        # so they don't delay the first x chunk behind them on HWDGE — they
        # aren't needed until the first down pair, ~3us into the phase.
        sd_all = swpool.tile([128, NF_S, H], dt.float8e4, tag="sd")
        nc.gpsimd.dma_start(sd_all[:], sd_s.rearrange("(ft p) h -> p ft h", p=128)[:])

        # routed weights (8MB fp8): tiles allocated now, DMAs deferred — they
        # are emitted paced across the shared phase (on the SWDGE queue) so
        # they don't steal HBM bandwidth from the shared phase's startup.
        w_dma_fns = []
        wu_t = w_up.rearrange("(kt p) f -> p kt f", p=128)
        wu_all = wpool.tile([128, KT_H, F], dt.float8e4, tag="wu")
        for kt in range(KT_H):
            w_dma_fns.append(
                lambda kt=kt: nc.gpsimd.dma_start(
                    wu_all[:, kt, :], wu_t[:, kt, :]
                )
            )
        wd_t = w_down.rearrange("(ft p) h -> p ft h", p=128)
        wd_all = wpool.tile([128, KT_F, H], dt.float8e4, tag="wd")
        for j in range(KT_F // 4):
            w_dma_fns.append(
                lambda j=j: nc.gpsimd.dma_start(
                    wd_all[:, 4 * j : 4 * j + 4, :], wd_t[:, 4 * j : 4 * j + 4, :]
                )
            )
        # prefetch of phase R's first x chunk and the gates, also paced
        # across phase S so phase R starts without waiting on DMA.
        RCHUNK = min(512, c_routed)
        xr0 = cpool.tile([128, KT_H, RCHUNK], dt.float8e4, tag="xr0")
        g_sb = cpool.tile([128, c_routed // 128], dt.float32, tag="g")
        w_dma_fns.append(lambda: nc.gpsimd.dma_start(g_sb[:], gates[:]))
        w_dma_fns.append(
            lambda: nc.gpsimd.dma_start(xr0[:], xTr_t[:, :, 0:RCHUNK])
        )

        # phase S: partial shared FFN over all tokens, F-slice FS
        # (bf16 up, fp8 down; host undoes the 256x down-weight scale)
        _ffn_phase(nc, tile, dt, act, up_fp8=False, wu=su, wd_all=sd_all,
                   x_r=xTs_t, out_r=outs_t, c_hi=t_total, n_f=NF_S,
                   pools=pools, chunk=256, paced_dmas=w_dma_fns,
                   front_dmas=su_front)

        # phase R: routed expert over gathered tokens, all fp8, gated
        # eviction; 512-token chunks hide the DoubleRow LDWEIGHTS.
        _ffn_phase(nc, tile, dt, act, up_fp8=True, wu=wu_all, wd_all=wd_all,
                   x_r=xTr_t, out_r=outr_t, c_hi=c_routed, n_f=KT_F,
                   pools=pools, chunk=RCHUNK, act_scale=1.0 / (SX * SW),
                   g_sb=g_sb, x0_pre=xr0)

    nc.finalize()
    return nc


def _get_nc(c_routed, t_total):
    key = (c_routed, t_total)
    if key not in _nc_cache:
        _nc_cache[key] = _build_nc(c_routed, t_total)
    return _nc_cache[key]


def _route(xf, router_w):
    """Host router in f64: top-2 indices (jax tie-break: lower index first)
    and their softmax probs."""
    logits = xf.astype(np.float64) @ router_w.astype(np.float64)
    m = logits.max(-1, keepdims=True)
    p = np.exp(logits - m)
    p /= p.sum(-1, keepdims=True)
    order = np.argsort(-p, axis=-1, kind="stable")
    top_idx = order[:, :TOPK]
    top_p = np.take_along_axis(p, top_idx, -1).astype(np.float32)
    return top_idx, top_p


def kernel(**inputs):
    x = np.ascontiguousarray(np.asarray(inputs["x"], np.float32))
    shared_up = np.asarray(inputs["shared_up"], np.float32)[0]
    shared_down = np.asarray(inputs["shared_down"], np.float32)[0]
    routed_up = np.asarray(inputs["routed_up"], np.float32)
    routed_down = np.asarray(inputs["routed_down"], np.float32)
    router_w = np.asarray(inputs["router_w"], np.float32)

    B, S, _ = x.shape
    T = B * S
    xf = x.reshape(T, H)

    top_idx, top_p = _route(xf, router_w)

    token_lists = [np.where((top_idx == e).any(-1))[0] for e in range(E)]
    c_cap = max(128, -(-max(len(l) for l in token_lists) // 128) * 128)

    # position of (token, slot) inside its expert's gathered buffer
    pos = np.zeros((T, TOPK), np.int64)
    gates_per_e = np.zeros((E, c_cap), np.float32)
    for e in range(E):
        lst = token_lists[e]
        for k in range(TOPK):
            sel = np.where(top_idx[:, k] == e)[0]
            p_in = np.searchsorted(lst, sel)
            pos[sel, k] = p_in
            gates_per_e[e, p_in] = top_p[sel, k]
    gates_per_e /= SW  # undo the fp8 down-weight scale at eviction

    xf_bf = xf.astype(BF16)
    xTs = np.ascontiguousarray(xf_bf.T)  # [H, T], shared phase input (bf16)
    xf_q = (xf * SX).astype(FP8)  # routed phase input (fp8, scaled)
    su_bf = shared_up.astype(BF16)
    sd_q = (shared_down * SW).astype(FP8)

    in_maps = []
    for e in range(E):
        lst = token_lists[e]
        xe = np.zeros((c_cap, H), FP8)
        xe[: len(lst)] = xf_q[lst]
        in_maps.append(
            {
                "xT_r": np.ascontiguousarray(xe.T),
                "xT_s": xTs,
                "gates": np.ascontiguousarray(
                    gates_per_e[e].reshape(c_cap // 128, 128).T
                ),
                "w_up": (routed_up[e] * SW).astype(FP8),
                "w_down": (routed_down[e] * SW).astype(FP8),
                "su_s": np.ascontiguousarray(su_bf[:, e * FS : (e + 1) * FS]),
                "sd_s": np.ascontiguousarray(sd_q[e * FS : (e + 1) * FS, :]),
            }
        )

    from concourse.bass_utils import run_bass_kernel_spmd

    nc = _get_nc(c_cap, T)
    res = run_bass_kernel_spmd(nc, in_maps, list(range(N_CORES)), trace=TRACE)
    global LAST_RESULT
    LAST_RESULT = res

    y = xf.copy()
    acc = np.zeros_like(xf)
    for e in range(E):
        acc += res.results[e]["out_s"].astype(np.float32)
    y += acc / SW  # undo the fp8 shared-down weight scale
    y_routed = np.stack(
        [res.results[e]["out_r"].astype(np.float32) for e in range(E)]
    )  # gated rows
    for k in range(TOPK):
        y += y_routed[top_idx[:, k], pos[:, k]]
    return y.reshape(B, S, H)
